# revision 4
# baseline (speedup 1.0000x reference)
"""AttentionMap kernel for 8 TRN2 NeuronCores.

Reference computation (B=2, L=512, T=1024, E=2560, H=128, HW=20):
    xe = x @ W_up.T                       [B,L,T] -> [B,L,E]
    t  = xe @ W_proj.T                    [B,L,E] -> [B,L,2E] -> [B,H,L,2*HW]
    q, k = split(t);  q *= HW**-0.5
    a  = q @ k.T + bias.transpose(0,3,1,2)
    a  = where(mask[b,None,None,lk], a, 0)
    out = a.transpose(0,2,3,1)            [B,Lq,Lk,H]

Strategy: shard the 128 heads across 8 cores (16 heads each), no collectives.
Per core, fold the two projections into one weight on device:
    W_combT[tau, f] = sum_e W_up[e, tau] * W_projsel[f, e]   (f = 640 cols: q|k)
    tT[f, r] = sum_tau W_combT[tau, f] * xT[tau, r]          (r = b*L + l)
    scores[lq, lk] = sum_d qT[d, lq] * kT[d, lk]             (d = head dim, 20)
The mask is folded host-side into a masked copy of x (used for the k half
only) and into the bias. The head dim is padded to stride 32 inside W_combT /
tT so per-head q/k slices are 32-aligned for PE row-tiling; pad lanes are
never read. All device compute is fp16 with fp32 PSUM accumulation.
"""

import numpy as np

import concourse.bass as bass
import concourse.mybir as mybir
import concourse.tile as tile
from concourse import bacc
from concourse.bass import ts
from concourse.bass_utils import run_bass_kernel_spmd

B, L, T = 2, 512, 1024
E, H = 2560, 128
HW = 20          # head width
NCORES = 8
HL = H // NCORES  # 16 local heads
PH = 32           # padded head stride
R = B * L         # 1024 token rows
FQ = HL * PH      # 512 padded q cols
F = 2 * FQ        # 1024 padded f dim
FU = HL * HW      # 320 unpadded q (or k) cols

f16 = mybir.dt.float16
f32 = mybir.dt.float32

KT_E = E // 128   # 20 contraction tiles over e
KT_T = T // 128   # 8 contraction tiles over tau

_BUILT = None


def _build():
    """Build the per-core Bass graph (same SPMD graph on all 8 cores)."""
    global _BUILT
    if _BUILT is not None:
        return _BUILT

    nc = bacc.Bacc(None, target_bir_lowering=False)

    xT_d = nc.dram_tensor("xT", [T, R], f16, kind="ExternalInput")
    xTm_d = nc.dram_tensor("xTm", [T, R], f16, kind="ExternalInput")
    wup_d = nc.dram_tensor("wup", [E, T], f16, kind="ExternalInput")
    wpT_d = nc.dram_tensor("wpT", [E, 2 * FU], f16, kind="ExternalInput")
    bias_d = nc.dram_tensor("biasP", [B, HL, L, L], f16, kind="ExternalInput")
    out_d = nc.dram_tensor("out", [B, HL, L, L], f16, kind="ExternalOutput")

    with tile.TileContext(nc) as tc:
        with (
            tc.tile_pool(name="win", bufs=1) as wpool,
            tc.tile_pool(name="xin", bufs=1) as xpool,
            tc.tile_pool(name="wc", bufs=1) as wcpool,
            tc.tile_pool(name="tt", bufs=1) as ttpool,
            tc.tile_pool(name="bias", bufs=4) as bpool,
            tc.tile_pool(name="outp", bufs=4) as opool,
        ):
            # ---- input loads (per 128-row tile so compute can start early)
            wup_t = []
            for k in range(KT_E):
                tl = wpool.tile([128, T], f16, tag=f"wup{k}", name=f"wup{k}")
                nc.sync.dma_start(tl[:], wup_d[ts(k, 128), :])
                wup_t.append(tl)
            wpT_t = []
            for k in range(KT_E):
                tl = wpool.tile([128, 2 * FU], f16, tag=f"wpT{k}", name=f"wpT{k}")
                nc.sync.dma_start(tl[:], wpT_d[ts(k, 128), :])
                wpT_t.append(tl)
            xT_t, xTm_t = [], []
            for k in range(KT_T):
                tl = xpool.tile([128, R], f16, tag=f"xT{k}", name=f"xT{k}")
                nc.sync.dma_start(tl[:], xT_d[ts(k, 128), :])
                xT_t.append(tl)
                tl = xpool.tile([128, R], f16, tag=f"xTm{k}", name=f"xTm{k}")
                nc.sync.dma_start(tl[:], xTm_d[ts(k, 128), :])
                xTm_t.append(tl)

            # ---- fold: W_combT[tau, f] (pad lanes uninitialized, never read)
            wc_t = []
            for m in range(KT_T):
                wc_t.append(wcpool.tile([128, F], f16, tag=f"wc{m}", name=f"wc{m}"))

            with tc.tile_pool(name="psA", bufs=2, space="PSUM") as psA:
                for m in range(KT_T):
                    for half in range(2):  # 0: q cols, 1: k cols
                        ps = psA.tile([128, FU], f32, tag=f"fold{half}", name=f"psfold{half}")
                        for k in range(KT_E):
                            nc.tensor.matmul(
                                ps[:],
                                wup_t[k][:, ts(m, 128)],
                                wpT_t[k][:, half * FU:(half + 1) * FU],
                                start=(k == 0),
                                stop=(k == KT_E - 1),
                            )
                        dst = (
                            wc_t[m][:, half * FQ:(half + 1) * FQ]
                            .rearrange("p (j d) -> p j d", d=PH)[:, :, 0:HW]
                        )
                        src = ps[:].rearrange("p (j d) -> p j d", d=HW)
                        nc.any.tensor_copy(dst, src)

            # ---- tT[f, r] = W_combT.T @ xT  (k half uses masked x)
            tT_t = [None] * KT_T
            with tc.tile_pool(name="psB", bufs=2, space="PSUM") as psB:
                for m in (0, 4, 1, 5, 2, 6, 3, 7):  # q/k pairs early per group
                    tl = ttpool.tile([128, R], f16, tag=f"tt{m}", name=f"tt{m}")
                    tT_t[m] = tl
                    rhs_t = xT_t if m < 4 else xTm_t
                    for n in range(2):
                        ps = psB.tile([128, 512], f32, tag="pst", name="pst")
                        for k in range(KT_T):
                            nc.tensor.matmul(
                                ps[:],
                                wc_t[k][:, ts(m, 128)],
                                rhs_t[k][:, ts(n, 512)],
                                start=(k == 0),
                                stop=(k == KT_T - 1),
                            )
                        nc.any.tensor_copy(tl[:, ts(n, 512)], ps[:])

                # ---- scores + bias + store, per (batch, local head)
                with tc.tile_pool(name="psC", bufs=3, space="PSUM") as psC:
                    for b in range(B):
                        for j in range(HL):
                            g, r32 = divmod(j, 4)
                            base = PH * r32
                            qt = tT_t[g]
                            kt = tT_t[4 + g]
                            bias_t = bpool.tile([128, 4 * L], f16, tag="bias", name="bias_t")
                            nc.sync.dma_start(
                                bias_t[:].rearrange("p (a k) -> p a k", a=4),
                                bias_d[b, j].rearrange("(a p) k -> p a k", p=128),
                            )
                            out_t = opool.tile([128, 4 * L], f16, tag="out", name="out_t")
                            for pair in range(2):
                                ps = psC.tile([128, 2 * L], f32, tag="sc", name="pssc")
                                for i in range(2):
                                    lqb = pair * 2 + i
                                    nc.tensor.matmul(
                                        ps[:, ts(i, L)],
                                        qt[base:base + HW,
                                           b * L + lqb * 128: b * L + (lqb + 1) * 128],
                                        kt[base:base + HW, b * L:(b + 1) * L],
                                        start=True,
                                        stop=True,
                                        tile_position=(base, 0),
                                    )
                                nc.any.tensor_add(
                                    out=out_t[:, ts(pair, 2 * L)],
                                    in0=ps[:],
                                    in1=bias_t[:, ts(pair, 2 * L)],
                                )
                            nc.sync.dma_start(
                                out_d[b, j].rearrange("(a p) k -> p a k", p=128),
                                out_t[:].rearrange("p (a k) -> p a k", a=4),
                            )

    nc.finalize()
    _BUILT = nc
    return nc


def _prep(x, mask, bias, W_up, W_proj):
    """Host-side sharding / layout prep (not part of timed device execution)."""
    xf = np.asarray(x, np.float32).reshape(R, T)
    maskv = np.asarray(mask).reshape(R).astype(np.float32)
    xT = np.ascontiguousarray(xf.T).astype(np.float16)
    xTm = (xT * maskv[None, :]).astype(np.float16)
    wup = np.asarray(W_up, np.float32).astype(np.float16)  # [E, T]

    Wp = np.asarray(W_proj, np.float32).reshape(H, 2 * HW, E)
    scale = np.float32(HW ** -0.5)
    mask_b = np.asarray(mask)  # [B, L] bool

    in_maps = []
    for c in range(NCORES):
        hs = slice(c * HL, (c + 1) * HL)
        wq = Wp[hs, :HW, :].reshape(FU, E) * scale   # [320, E]
        wk = Wp[hs, HW:, :].reshape(FU, E)           # [320, E]
        wpT = np.ascontiguousarray(
            np.concatenate([wq, wk], axis=0).T
        ).astype(np.float16)                         # [E, 640]
        bs = np.transpose(bias[:, :, :, hs], (0, 3, 1, 2))  # [B, HL, L, L]
        bs = np.where(mask_b[:, None, None, :], bs, 0.0).astype(np.float16)
        in_maps.append({
            "xT": xT,
            "xTm": xTm,
            "wup": wup,
            "wpT": wpT,
            "biasP": np.ascontiguousarray(bs),
        })
    return in_maps


def _assemble(core_results):
    shards = [np.asarray(r["out"]) for r in core_results]  # [B, HL, L, L] f16
    out = np.stack(shards, axis=0)                  # [NC, B, HL, L, L]
    out = out.transpose(1, 3, 4, 0, 2)              # [B, Lq, Lk, NC, HL]
    return np.ascontiguousarray(out.reshape(B, L, L, H).astype(np.float32))


def _run(inputs, trace=False, **kwargs):
    nc = _build()
    in_maps = _prep(**inputs)
    res = run_bass_kernel_spmd(
        nc, in_maps, core_ids=list(range(NCORES)), trace=trace, **kwargs
    )
    return _assemble(res.results), res


def kernel(**inputs):
    out, _ = _run(inputs, trace=False)
    return out
